# revision 37
# baseline (speedup 1.0000x reference)
"""Distributed Trainium2 kernel for nn_Attention_79207786873592.

Full attention block: qkv projection -> RMSNorm(q,k) -> RoPE -> SDPA -> wo.
B=4, L=2048, D=2048, H=16 heads, head_dim=128, fp32 I/O (bf16 compute).

Sharding: 8-way tensor-parallel over heads (2 heads/core), as in v1.

v2: software-pipelined fusion. The attention phase is ACT-paced (exp is
~780ns per [128,512] tile while the two PE matmuls of the same iteration
only need ~430ns), so the PE-bound qkv projection of batch b+1 and the
wo projection of batch b-1 are emitted interleaved into attention(b)'s
j-loop. Generators yield between work units; a static scheduler injects
qkv/wo units at attention yield points so every engine stays busy.

Other changes vs v1:
  - v is computed directly in [token, dout] layout (x-block stationary,
    wqkv-v columns moving), eliminating the PE transposes.
  - softmax tails use reciprocal_approx_fast (DVE custom op, ~670ns) in
    place of RECIPROCAL (~4us).
  - AllGather is split per (batch, head-pair) halves for earlier overlap;
    wo weights are host-permuted to [even heads, odd heads] ctx order.
  - PSUM banks: qk-proj 2, v-proj 1, scores/misc 3 (shared ring), ctx 2.
"""

import sys

sys.path.insert(0, "/opt/trn_rl_repo")

import numpy as np
import ml_dtypes

import concourse.bass as bass
import concourse.tile as tile
import concourse.mybir as mybir
from concourse import bacc

B, L, D, H = 4, 2048, 2048, 16
HD = D // H              # 128
NC = 8                   # cores
HPC = H // NC            # 2 heads per core
DQ = HPC * HD            # 256 rows of q/k/v per core
T = B * L                # 8192 tokens
EPS = 1e-5
CH = 512                 # token chunk
NCPB = L // CH           # 4 chunks per batch
NDT = D // 128           # 16 contraction tiles
NJ = L // 128            # 16 k-tiles per batch
IW = 512                 # q-position chunk width
BF = mybir.dt.bfloat16
F32 = mybir.dt.float32
BF_NP = ml_dtypes.bfloat16

_CACHE = {}


class Feeder:
    """Emits units of a generator at scheduled spine-yield slots."""

    def __init__(self, gen, n_units, first, last):
        self.gen = gen
        self.done = False
        if n_units > 0:
            span = max(1, last - first)
            self.slots = [first + (k * span) // max(1, n_units - 1)
                          if n_units > 1 else first
                          for k in range(n_units)]
        else:
            self.slots = []
        self.idx = 0

    def step(self, slot):
        while (not self.done and self.idx < len(self.slots)
               and self.slots[self.idx] <= slot):
            try:
                next(self.gen)
                self.idx += 1
            except StopIteration:
                self.done = True

    def drain(self):
        while not self.done:
            try:
                next(self.gen)
            except StopIteration:
                self.done = True


def build_nc():
    nc = bacc.Bacc("TRN2", target_bir_lowering=False, debug=False, num_devices=NC)

    # pre-blocked on host: xQ[c] is exactly one chunk's SBUF image
    # (16KB contiguous per partition), wqQ/woQ are the SBUF weight images
    xQ = nc.dram_tensor("xQ", [B * NCPB, 128, NDT * CH], BF,
                        kind="ExternalInput").ap()
    wqQ = nc.dram_tensor("wqQ", [128, NDT * 3 * DQ], BF,
                         kind="ExternalInput").ap()
    woQ = nc.dram_tensor("woQ", [128, NDT * DQ], BF,
                         kind="ExternalInput").ap()
    TAB_NAMES = [p + sfx for p in ("tq", "tk")
                 for sfx in ("ce", "so", "co", "se")]
    tab_ext = {nm: nc.dram_tensor(nm, [HD, L], BF, kind="ExternalInput").ap()
               for nm in TAB_NAMES}
    outT = nc.dram_tensor("outT", [DQ, T], F32, kind="ExternalOutput").ap()

    with tile.TileContext(nc) as tc:
        with tc.tile_pool(name="dram", bufs=1, space="DRAM") as dram, \
             tc.tile_pool(name="consts", bufs=1) as consts, \
             tc.tile_pool(name="wres", bufs=1) as wres:

            q_dram = dram.tile([DQ, T], BF, tag="q_dram")
            k_dram = dram.tile([DQ, T], BF, tag="k_dram")
            v_dram = dram.tile([T, DQ], BF, tag="v_dram")
            ss_in = [dram.tile([2, L], F32, tag=f"ss_in{b}", name=f"ss_in{b}")
                     for b in range(B)]
            ss_out = [dram.tile([2, L], F32, tag=f"ss_out{b}", name=f"ss_out{b}",
                                addr_space="Shared") for b in range(B)]
            ss_fin = [dram.tile([2, L], F32, tag=f"ss_fin{b}", name=f"ss_fin{b}")
                      for b in range(B)]
            ss_finb = [dram.tile([2, L], BF, tag=f"ss_finb{b}",
                                 name=f"ss_finb{b}") for b in range(B)]
            ctx_bh = [[dram.tile([128, L], BF, tag=f"ctxb{b}h{hl}",
                                 name=f"ctxb{b}h{hl}") for hl in range(2)]
                      for b in range(B)]
            ctx_gh = [[dram.tile([NC * 128, L], BF, tag=f"ctxg{b}h{hl}",
                                 name=f"ctxg{b}h{hl}", addr_space="Shared")
                       for hl in range(2)] for b in range(B)]
            # batch 3 / hl 1: column-split halves so the final AllGather
            # exposes only half the transfer after the last attention group
            ctx_b31 = [dram.tile([128, L // 2], BF, tag=f"ctxb31{ii}",
                                 name=f"ctxb31{ii}") for ii in range(2)]
            ctx_g31 = [dram.tile([NC * 128, L // 2], BF, tag=f"ctxg31{ii}",
                                 name=f"ctxg31{ii}", addr_space="Shared")
                       for ii in range(2)]

            ones_cb = consts.tile([128, 1], BF, tag="ones_cb")
            nc.vector.memset(ones_cb[:], 1.0)
            ones_r = consts.tile([1, 128], F32, tag="ones_r")
            nc.vector.memset(ones_r[:], 1.0)
            ones_rb = consts.tile([1, 128], BF, tag="ones_rb")
            nc.vector.memset(ones_rb[:], 1.0)
            eps2 = consts.tile([128, 1], F32, tag="eps2")
            nc.vector.memset(eps2[:], EPS)
            # ln(sqrt(sc2)): fold of 1/sqrt(HD) for q rows into the exp-form
            # rsqrt (avoids the Sqrt ACT table set entirely)
            lnsc = consts.tile([128, 1], F32, tag="lnsc")
            nc.vector.memset(lnsc[0:64, :], -0.5 * float(np.log(HD)))
            nc.vector.memset(lnsc[64:128, :], 0.0)

            # resident weights + rope tables (host pre-blocked: one
            # contiguous run per partition). Only wq is loaded here; the
            # first x chunk's DMA is emitted next (critical path), then
            # wo/tabs follow.
            wq_sb = wres.tile([128, NDT * 3 * DQ], BF, tag="wq_sb")
            nc.sync.dma_start(wq_sb[:], wqQ[:, :])
            wo_sb = wres.tile([128, NDT * DQ], BF, tag="wo_sb")
            tabs = {}

            def load_rest_of_weights():
                nc.sync.dma_start(wo_sb[:], woQ[:, :])
                for nm, t in tab_ext.items():
                    tt = wres.tile([128, L], BF, tag=nm + "_sb",
                                   name=nm + "_sb")
                    nc.sync.dma_start(tt[:], t[:, :])
                    tabs[nm] = tt

            v_dr = v_dram.rearrange("(a p) q -> p a q", p=128)

            with tc.tile_pool(name="xcp", bufs=2) as xcp, \
                 tc.tile_pool(name="p1sb", bufs=2) as p1sb, \
                 tc.tile_pool(name="qkp", bufs=2, space="PSUM") as qkp, \
                 tc.tile_pool(name="vpp", bufs=1, space="PSUM") as vpp, \
                 tc.tile_pool(name="sp", bufs=3, space="PSUM") as sp, \
                 tc.tile_pool(name="cxp", bufs=2, space="PSUM") as cxp, \
                 tc.tile_pool(name="att", bufs=2) as att, \
                 tc.tile_pool(name="ptp", bufs=3) as ptp, \
                 tc.tile_pool(name="dap", bufs=2) as dap, \
                 tc.tile_pool(name="tlp", bufs=2) as tlp, \
                 tc.tile_pool(name="wsb", bufs=2) as wsb:

                # ---------------- qkv projection units ----------------
                def emit_v_pair(c, xc, tbp):
                    vp = vpp.tile([128, 512], F32, tag="vp", name="vp")
                    for half in range(2):
                        tb_ = tbp * 2 + half
                        hsl = slice(half * 256, (half + 1) * 256)
                        for d in range(NDT):
                            nc.tensor.matmul(
                                vp[:, hsl],
                                xc[:, d * CH + tb_ * 128:
                                   d * CH + (tb_ + 1) * 128],
                                wq_sb[:, d * 3 * DQ + 2 * DQ:
                                      d * 3 * DQ + 3 * DQ],
                                start=(d == 0), stop=(d == NDT - 1))
                    vsb = p1sb.tile([128, 512], BF, tag="vsb")
                    nc.scalar.copy(vsb[:], vp[:])
                    r0 = c * CH + tbp * 256
                    nc.sync.dma_start(v_dram[r0:r0 + 128, :], vsb[:, 0:256])
                    nc.sync.dma_start(v_dram[r0 + 128:r0 + 256, :],
                                      vsb[:, 256:512])

                def emit_ar_ssf(b):
                    # AllReduce sumsq, compute 1/rms (exp/ln form: stays in
                    # the natural_log_exp activation table set)
                    nc.gpsimd.collective_compute(
                        "AllReduce", mybir.AluOpType.add,
                        replica_groups=[list(range(NC))],
                        ins=[ss_in[b].opt()], outs=[ss_out[b].opt()])
                    FW = 2 * L // 128
                    ssf = p1sb.tile([128, FW], F32, tag="ssf")
                    nc.sync.dma_start(
                        ssf[:],
                        ss_out[b].rearrange("a (p f) -> (a p) f", p=64)[:, :])
                    nc.scalar.activation(ssf[:], ssf[:],
                                         mybir.ActivationFunctionType.Ln,
                                         bias=eps2[:], scale=1.0 / D)
                    nc.scalar.activation(ssf[:], ssf[:],
                                         mybir.ActivationFunctionType.Exp,
                                         bias=lnsc[:], scale=-0.5)
                    nc.sync.dma_start(
                        ss_fin[b].rearrange("a (p f) -> (a p) f", p=64)[:, :],
                        ssf[:])
                    ssfb = p1sb.tile([128, FW], BF, tag="ssfb")
                    nc.vector.tensor_copy(ssfb[:], ssf[:])
                    nc.sync.dma_start(
                        ss_finb[b].rearrange("a (p f) -> (a p) f", p=64)[:, :],
                        ssfb[:])

                def qkv_work(b):
                    for lc in range(NCPB):
                        c = b * NCPB + lc
                        csl = slice(c * CH, (c + 1) * CH)
                        tsl = slice(lc * CH, (lc + 1) * CH)
                        xc = xcp.tile([128, NDT * CH], BF, tag="xc")
                        nc.sync.dma_start(xc[:], xQ[c])
                        yield
                        for side in range(2):    # 0=q, 1=k
                            pscs = []
                            sqs_ = []
                            for eo in range(2):  # 0=evens, 1=odds
                                m = side * 2 + eo
                                ps = qkp.tile([128, CH], F32, tag="qk",
                                              name="qk")
                                for d in range(NDT):
                                    nc.tensor.matmul(
                                        ps[:],
                                        wq_sb[:, d * 3 * DQ + m * 128:
                                              d * 3 * DQ + (m + 1) * 128],
                                        xc[:, d * CH:(d + 1) * CH],
                                        start=(d == 0), stop=(d == NDT - 1))
                                sq = p1sb.tile([128, CH], BF, tag="sq",
                                               name="sq", bufs=3)
                                nc.scalar.square(sq[:], ps[:])
                                sqs_.append(sq)
                                psc = p1sb.tile([128, CH], BF, tag="psc",
                                                name="psc", bufs=3)
                                nc.scalar.copy(psc[:], ps[:])
                                pscs.append(psc)
                                yield
                            ssp = sp.tile([128, CH], F32, tag="sp", name="ssp")
                            for eo in range(2):
                                nc.tensor.matmul(ssp[0:1, :], ones_cb[:],
                                                 sqs_[eo][:],
                                                 start=(eo == 0),
                                                 stop=(eo == 1))
                            ssr = p1sb.tile([1, CH], F32, tag="ssr", bufs=1)
                            nc.scalar.copy(ssr[:], ssp[0:1, :])
                            nc.sync.dma_start(
                                ss_in[b][side:side + 1, tsl], ssr[:])
                            # rope (norm weights folded into tables on host)
                            psE, psO = pscs
                            pre = "tq" if side == 0 else "tk"
                            ta = p1sb.tile([128, CH], BF, tag="ta", bufs=1)
                            nc.vector.tensor_mul(ta[:], psE[:],
                                                 tabs[pre + "ce"][:, tsl])
                            tb = p1sb.tile([128, CH], BF, tag="tb", bufs=1)
                            nc.vector.tensor_mul(tb[:], psO[:],
                                                 tabs[pre + "so"][:, tsl])
                            roE = p1sb.tile([128, CH], BF, tag="roE")
                            nc.vector.tensor_sub(roE[:], ta[:], tb[:])
                            tc_ = p1sb.tile([128, CH], BF, tag="tc_", bufs=1)
                            nc.vector.tensor_mul(tc_[:], psO[:],
                                                 tabs[pre + "co"][:, tsl])
                            td = p1sb.tile([128, CH], BF, tag="td", bufs=1)
                            nc.vector.tensor_mul(td[:], psE[:],
                                                 tabs[pre + "se"][:, tsl])
                            roO = p1sb.tile([128, CH], BF, tag="roO")
                            nc.vector.tensor_add(roO[:], tc_[:], td[:])
                            # q/k stored in PSUM row order: [evens h0, evens
                            # h1, odds h0, odds h1]
                            dst = q_dram if side == 0 else k_dram
                            nc.sync.dma_start(dst[0:128, csl], roE[:])
                            nc.sync.dma_start(dst[128:256, csl], roO[:])
                            yield
                        if lc < NCPB - 1:
                            for tbp in range(2):
                                emit_v_pair(c, xc, tbp)
                                yield
                        else:
                            # last chunk: AllReduce first, v work covers its
                            # latency
                            emit_ar_ssf(b)
                            yield
                            for tbp in range(2):
                                emit_v_pair(c, xc, tbp)
                                yield

                QKV_UNITS = NCPB * 9 + 1   # 37

                # ---------------- wo projection units ----------------
                def wo_work(b):
                    for i in range(L // 512):
                        cg = wsb.tile([128, NDT * 512], BF, tag="cg")
                        cgr = cg.rearrange("p (a t) -> p a t", a=NDT)
                        isl = slice(i * 512, (i + 1) * 512)
                        nc.sync.dma_start(
                            cgr[:, 0:8, :],
                            ctx_gh[b][0].rearrange("(a p) t -> p a t",
                                                   p=128)[:, :, isl])
                        if b == B - 1:
                            half = i // 2
                            co = i * 512 - half * 1024
                            nc.sync.dma_start(
                                cgr[:, 8:16, :],
                                ctx_g31[half].rearrange(
                                    "(a p) t -> p a t",
                                    p=128)[:, :, co:co + 512])
                        else:
                            nc.sync.dma_start(
                                cgr[:, 8:16, :],
                                ctx_gh[b][1].rearrange("(a p) t -> p a t",
                                                       p=128)[:, :, isl])
                        yield
                        for m in range(DQ // 128):
                            op = sp.tile([128, 512], F32, tag="sp", name="wop")
                            for d in range(NDT):
                                nc.tensor.matmul(
                                    op[:],
                                    wo_sb[:, d * DQ + m * 128:
                                          d * DQ + (m + 1) * 128],
                                    cg[:, d * 512:(d + 1) * 512],
                                    start=(d == 0), stop=(d == NDT - 1))
                            osb = p1sb.tile([128, 512], F32, tag="osb")
                            nc.scalar.copy(osb[:], op[:])
                            nc.sync.dma_start(
                                outT[m * 128:(m + 1) * 128,
                                     b * L + i * 512:b * L + (i + 1) * 512],
                                osb[:])
                            yield

                WO_UNITS = (L // 512) * 3  # 12

                # ---------------- attention spine ----------------
                def attn_spine(b):
                    bsl = slice(b * L, (b + 1) * L)
                    sqs = att.tile([1, L], BF, tag="sqs", bufs=1)
                    nc.sync.dma_start(sqs[:], ss_finb[b][0:1, :])
                    skc = att.tile([128, NJ], F32, tag="skc")
                    nc.sync.dma_start(
                        skc[:],
                        ss_fin[b][1:2, :].rearrange("a (j p) -> (a p) j",
                                                    p=128))
                    yield
                    for hl in range(2):
                        qn = att.tile([128, L], BF, tag="qn")
                        nc.sync.dma_start(
                            qn[0:64, :], q_dram[hl * 64:(hl + 1) * 64, bsl])
                        nc.sync.dma_start(
                            qn[64:128, :],
                            q_dram[128 + hl * 64:128 + (hl + 1) * 64, bsl])
                        kn = att.tile([128, L], BF, tag="kn")
                        nc.sync.dma_start(
                            kn[0:64, :], k_dram[hl * 64:(hl + 1) * 64, bsl])
                        nc.sync.dma_start(
                            kn[64:128, :],
                            k_dram[128 + hl * 64:128 + (hl + 1) * 64, bsl])
                        vh = att.tile([128, NJ * 128], BF, tag="vh")
                        nc.sync.dma_start(
                            vh.rearrange("p (a q) -> p a q", a=NJ),
                            v_dr[:, b * NJ:(b + 1) * NJ,
                                 hl * 128:(hl + 1) * 128])
                        for ii in range(L // 512):
                            isl = slice(ii * 512, (ii + 1) * 512)
                            bp = sp.tile([128, 512], F32, tag="sp", name="bp")
                            nc.tensor.matmul(
                                bp[:], ones_rb[:], sqs[0:1, isl],
                                start=True, stop=True)
                            nc.vector.tensor_mul(qn[:, isl], qn[:, isl],
                                                 bp[:])
                        yield
                        for g in range(L // (2 * IW)):
                            ics = (2 * g, 2 * g + 1)
                            cps = {}
                            dacc = {}
                            for ic in ics:
                                cps[ic] = cxp.tile([128, IW], F32, tag="cx",
                                                   name=f"cx{ic % 2}")
                                dacc[ic] = [dap.tile([128, IW], BF,
                                                     tag=f"da{ic % 2}{e}",
                                                     name=f"da{ic % 2}{e}")
                                            for e in range(2)]
                            for j in range(NJ):
                                for ic in ics:
                                    sps = sp.tile([128, IW], F32, tag="sp",
                                                  name="sps")
                                    nc.tensor.matmul(
                                        sps[:],
                                        kn[:, j * 128:(j + 1) * 128],
                                        qn[:, ic * IW:(ic + 1) * IW],
                                        start=True, stop=True)
                                    pt = ptp.tile([128, IW], BF, tag="pt",
                                                  name="pt")
                                    nc.scalar.activation(
                                        pt[:], sps[:],
                                        mybir.ActivationFunctionType.Exp,
                                        scale=skc[:, j:j + 1])
                                    da = dacc[ic][j % 2]
                                    if j < 2:
                                        nc.vector.tensor_copy(da[:], pt[:])
                                    else:
                                        nc.vector.tensor_add(da[:], da[:],
                                                             pt[:])
                                    nc.tensor.matmul(
                                        cps[ic][:],
                                        vh[:, j * 128:(j + 1) * 128],
                                        pt[:],
                                        start=(j == 0), stop=(j == NJ - 1))
                                yield
                            for ic in ics:
                                df = tlp.tile([128, IW], BF, tag="df")
                                nc.vector.tensor_add(df[:], dacc[ic][0][:],
                                                     dacc[ic][1][:])
                                dps = sp.tile([128, 512], F32, tag="sp",
                                              name="dps")
                                nc.tensor.matmul(dps[0:1, :], ones_cb[:],
                                                 df[:], start=True, stop=True)
                                rrow = tlp.tile([1, 512], F32, tag="rrow")
                                nc.vector.reciprocal_approx_fast(
                                    rrow[:], dps[0:1, :])
                                rrb = tlp.tile([1, 512], BF, tag="rrb")
                                nc.vector.tensor_copy(rrb[:], rrow[:])
                                rbp = sp.tile([128, 512], F32, tag="sp",
                                              name="rbp")
                                nc.tensor.matmul(rbp[:], ones_rb[:], rrb[:],
                                                 start=True, stop=True)
                                rbs = tlp.tile([128, 512], BF, tag="rbs")
                                nc.scalar.copy(rbs[:], rbp[:])
                                csb = tlp.tile([128, 512], BF, tag="csb")
                                nc.vector.tensor_mul(csb[:], cps[ic][:],
                                                     rbs[:])
                                if b == B - 1 and hl == 1:
                                    nc.sync.dma_start(
                                        ctx_b31[g][:, (ic - 2 * g) * IW:
                                                   (ic - 2 * g + 1) * IW],
                                        csb[:])
                                else:
                                    nc.sync.dma_start(
                                        ctx_bh[b][hl][:,
                                                      ic * IW:(ic + 1) * IW],
                                        csb[:])
                                yield
                            if b == B - 1 and hl == 1:
                                nc.gpsimd.collective_compute(
                                    "AllGather", mybir.AluOpType.bypass,
                                    replica_groups=[list(range(NC))],
                                    ins=[ctx_b31[g].opt()],
                                    outs=[ctx_g31[g].opt()])
                                yield
                        if not (b == B - 1 and hl == 1):
                            nc.gpsimd.collective_compute(
                                "AllGather", mybir.AluOpType.bypass,
                                replica_groups=[list(range(NC))],
                                ins=[ctx_bh[b][hl].opt()],
                                outs=[ctx_gh[b][hl].opt()])
                            yield

                # spine yields per batch: 1 + 2*(1 + 2*(16+2) + 1) = 77
                SPINE_SLOTS = 77

                qkv_gens = [qkv_work(b) for b in range(B)]
                wo_gens = [wo_work(b) for b in range(B)]

                # prologue: xc0 DMA first (critical path), then remaining
                # resident weights, then project batch 0 alone
                next(qkv_gens[0])
                load_rest_of_weights()
                f0 = Feeder(qkv_gens[0], 0, 0, 0)
                f0.gen = qkv_gens[0]
                f0.done = False
                f0.drain()

                for b in range(B):
                    feeders = []
                    if b + 1 < B:
                        feeders.append(
                            Feeder(qkv_gens[b + 1], QKV_UNITS, 2, 52))
                    if b >= 1:
                        feeders.append(
                            Feeder(wo_gens[b - 1], WO_UNITS, 22, SPINE_SLOTS - 6))
                    slot = 0
                    for _ in attn_spine(b):
                        slot += 1
                        for f in feeders:
                            f.step(slot)
                    for f in feeders:
                        f.drain()

                fN = Feeder(wo_gens[B - 1], 0, 0, 0)
                fN.gen = wo_gens[B - 1]
                fN.done = False
                fN.drain()

    nc.compile()
    return nc


def _prep_inputs(x_BLD, freqs, wqkv, wo, q_norm_w, k_norm_w):
    """Host-side sharding/layout. Returns in_maps (list of 8 dicts)."""
    x = np.asarray(x_BLD, np.float32)
    freqs = np.asarray(freqs, np.float32)
    wqkv = np.asarray(wqkv, np.float32)
    wo = np.asarray(wo, np.float32)
    qw = np.asarray(q_norm_w, np.float32)
    kw = np.asarray(k_norm_w, np.float32)

    # xQ[c] = chunk c's SBUF image: [128, a*CH + t] = x[c*CH+t, a*128+p]
    xQ = np.ascontiguousarray(
        x.reshape(T, D).astype(BF_NP)
        .reshape(B * NCPB, CH, NDT, 128).transpose(0, 3, 2, 1)
        .reshape(B * NCPB, 128, NDT * CH))
    sinT = np.ascontiguousarray(freqs[0].T)  # [D/2, L]
    cosT = np.ascontiguousarray(freqs[1].T)

    evens = 2 * np.arange(64)
    odds = evens + 1
    # ctx order after split AllGather: all even heads, then all odd heads
    woperm = np.concatenate(
        [h * HD + np.arange(HD) for h in range(0, H, 2)]
        + [h * HD + np.arange(HD) for h in range(1, H, 2)])

    in_maps = []
    for r in range(NC):
        heads = [HPC * r + hl for hl in range(HPC)]
        # q/k row order: [h0 evens, h1 evens, h0 odds, h1 odds]
        qrows = np.concatenate([h * HD + evens for h in heads]
                               + [h * HD + odds for h in heads])
        rows = np.concatenate([qrows, D + qrows, 2 * D + DQ * r + np.arange(DQ)])
        wqkvT = wqkv[rows, :].T.astype(BF_NP)          # [D, 3DQ]
        wqQ = np.ascontiguousarray(
            wqkvT.reshape(NDT, 128, 3 * DQ).transpose(1, 0, 2)
            .reshape(128, NDT * 3 * DQ))
        woT = wo[DQ * r:DQ * (r + 1), :][:, woperm].T.astype(BF_NP)
        woQ = np.ascontiguousarray(
            woT.reshape(NDT, 128, DQ).transpose(1, 0, 2)
            .reshape(128, NDT * DQ))

        tabs = {p + sfx: np.empty((HD, L), np.float32)
                for p in ("tq", "tk") for sfx in ("ce", "so", "co", "se")}
        for hl, h in enumerate(heads):
            rsl = slice(hl * 64, (hl + 1) * 64)
            cosP = cosT[h * 64:(h + 1) * 64]
            sinP = sinT[h * 64:(h + 1) * 64]
            for w, p in ((qw, "tq"), (kw, "tk")):
                w_e = w[h * HD + evens][:, None]
                w_o = w[h * HD + odds][:, None]
                tabs[p + "ce"][rsl] = w_e * cosP
                tabs[p + "so"][rsl] = w_o * sinP
                tabs[p + "co"][rsl] = w_o * cosP
                tabs[p + "se"][rsl] = w_e * sinP

        in_maps.append({
            "xQ": xQ,
            "wqQ": wqQ,
            "woQ": woQ,
            **{k: v.astype(BF_NP) for k, v in tabs.items()},
        })
    return in_maps


def _assemble(results):
    outT = np.empty((D, T), np.float32)
    for r in range(NC):
        outT[DQ * r:DQ * (r + 1)] = results[r]["outT"]
    return np.ascontiguousarray(outT.T).reshape(B, L, D)


def _install_ntff_hook():
    """The agent image's antenv lacks axon_hooks; provide the documented shim
    so run_bass_kernel_spmd(trace=True) can NTFF-profile via libaxon_pjrt."""
    try:
        import antenv.axon_hooks  # noqa: F401
        return
    except ImportError:
        pass
    import types
    hookf = None
    try:
        from trn_agent_boot.trn_boot import _ntff_profile_via_ctypes
        hookf = _ntff_profile_via_ctypes("/opt/axon/libaxon_pjrt.so")
    except Exception:
        pass
    mod = types.ModuleType("antenv.axon_hooks")
    state = {"h": hookf}
    mod.set_axon_ntff_profile_hook = lambda h: state.__setitem__("h", h)
    mod.get_axon_ntff_profile_hook = lambda: state["h"]
    sys.modules["antenv.axon_hooks"] = mod
    import antenv
    antenv.axon_hooks = mod


def kernel(x_BLD, freqs, wqkv, wo, q_norm_w, k_norm_w, _trace=False):
    from concourse.bass_utils import run_bass_kernel_spmd
    if _trace:
        _install_ntff_hook()
    if "nc" not in _CACHE:
        _CACHE["nc"] = build_nc()
    nc = _CACHE["nc"]
    in_maps = _prep_inputs(x_BLD, freqs, wqkv, wo, q_norm_w, k_norm_w)
    res = run_bass_kernel_spmd(nc, in_maps, core_ids=list(range(NC)),
                               trace=_trace)
    out = _assemble(res.results)
    if _trace:
        return out, res
    return out


# revision 38
# speedup vs baseline: 1.0037x; 1.0037x over previous
"""Distributed Trainium2 kernel for nn_Attention_79207786873592.

Full attention block: qkv projection -> RMSNorm(q,k) -> RoPE -> SDPA -> wo.
B=4, L=2048, D=2048, H=16 heads, head_dim=128, fp32 I/O (bf16 compute).

Sharding: 8-way tensor-parallel over heads (2 heads/core), as in v1.

v2: software-pipelined fusion. The attention phase is ACT-paced (exp is
~780ns per [128,512] tile while the two PE matmuls of the same iteration
only need ~430ns), so the PE-bound qkv projection of batch b+1 and the
wo projection of batch b-1 are emitted interleaved into attention(b)'s
j-loop. Generators yield between work units; a static scheduler injects
qkv/wo units at attention yield points so every engine stays busy.

Other changes vs v1:
  - v is computed directly in [token, dout] layout (x-block stationary,
    wqkv-v columns moving), eliminating the PE transposes.
  - softmax tails use reciprocal_approx_fast (DVE custom op, ~670ns) in
    place of RECIPROCAL (~4us).
  - AllGather is split per (batch, head-pair) halves for earlier overlap;
    wo weights are host-permuted to [even heads, odd heads] ctx order.
  - PSUM banks: qk-proj 2, v-proj 1, scores/misc 3 (shared ring), ctx 2.
"""

import sys

sys.path.insert(0, "/opt/trn_rl_repo")

import numpy as np
import ml_dtypes

import concourse.bass as bass
import concourse.tile as tile
import concourse.mybir as mybir
from concourse import bacc

B, L, D, H = 4, 2048, 2048, 16
HD = D // H              # 128
NC = 8                   # cores
HPC = H // NC            # 2 heads per core
DQ = HPC * HD            # 256 rows of q/k/v per core
T = B * L                # 8192 tokens
EPS = 1e-5
CH = 512                 # token chunk
NCPB = L // CH           # 4 chunks per batch
NDT = D // 128           # 16 contraction tiles
NJ = L // 128            # 16 k-tiles per batch
IW = 512                 # q-position chunk width
BF = mybir.dt.bfloat16
F32 = mybir.dt.float32
BF_NP = ml_dtypes.bfloat16

_CACHE = {}


class Feeder:
    """Emits units of a generator at scheduled spine-yield slots."""

    def __init__(self, gen, n_units, first, last):
        self.gen = gen
        self.done = False
        if n_units > 0:
            span = max(1, last - first)
            self.slots = [first + (k * span) // max(1, n_units - 1)
                          if n_units > 1 else first
                          for k in range(n_units)]
        else:
            self.slots = []
        self.idx = 0

    def step(self, slot):
        while (not self.done and self.idx < len(self.slots)
               and self.slots[self.idx] <= slot):
            try:
                next(self.gen)
                self.idx += 1
            except StopIteration:
                self.done = True

    def drain(self):
        while not self.done:
            try:
                next(self.gen)
            except StopIteration:
                self.done = True


def build_nc():
    nc = bacc.Bacc("TRN2", target_bir_lowering=False, debug=False, num_devices=NC)

    # pre-blocked on host: xQ[c] is exactly one chunk's SBUF image
    # (16KB contiguous per partition), wqQ/woQ are the SBUF weight images
    xQ = nc.dram_tensor("xQ", [B * NCPB, 128, NDT * CH], BF,
                        kind="ExternalInput").ap()
    wqQ = nc.dram_tensor("wqQ", [128, NDT * 3 * DQ], BF,
                         kind="ExternalInput").ap()
    woQ = nc.dram_tensor("woQ", [128, NDT * DQ], BF,
                         kind="ExternalInput").ap()
    TAB_NAMES = [p + sfx for p in ("tq", "tk")
                 for sfx in ("ce", "so", "co", "se")]
    tab_ext = {nm: nc.dram_tensor(nm, [HD, L], BF, kind="ExternalInput").ap()
               for nm in TAB_NAMES}
    outT = nc.dram_tensor("outT", [DQ, T], F32, kind="ExternalOutput").ap()

    with tile.TileContext(nc) as tc:
        with tc.tile_pool(name="dram", bufs=1, space="DRAM") as dram, \
             tc.tile_pool(name="consts", bufs=1) as consts, \
             tc.tile_pool(name="wres", bufs=1) as wres:

            q_dram = dram.tile([DQ, T], BF, tag="q_dram")
            k_dram = dram.tile([DQ, T], BF, tag="k_dram")
            v_dram = dram.tile([T, DQ], BF, tag="v_dram")
            ss_in = [dram.tile([2, L], F32, tag=f"ss_in{b}", name=f"ss_in{b}")
                     for b in range(B)]
            ss_out = [dram.tile([2, L], F32, tag=f"ss_out{b}", name=f"ss_out{b}",
                                addr_space="Shared") for b in range(B)]
            ss_fin = [dram.tile([2, L], F32, tag=f"ss_fin{b}", name=f"ss_fin{b}")
                      for b in range(B)]
            ss_finb = [dram.tile([2, L], BF, tag=f"ss_finb{b}",
                                 name=f"ss_finb{b}") for b in range(B)]
            ctx_bh = [[dram.tile([128, L], BF, tag=f"ctxb{b}h{hl}",
                                 name=f"ctxb{b}h{hl}") for hl in range(2)]
                      for b in range(B)]
            ctx_gh = [[dram.tile([NC * 128, L], BF, tag=f"ctxg{b}h{hl}",
                                 name=f"ctxg{b}h{hl}", addr_space="Shared")
                       for hl in range(2)] for b in range(B)]
            # batch 3 / hl 1: column-split halves so the final AllGather
            # exposes only half the transfer after the last attention group
            ctx_b31 = [dram.tile([128, L // 2], BF, tag=f"ctxb31{ii}",
                                 name=f"ctxb31{ii}") for ii in range(2)]
            ctx_g31 = [dram.tile([NC * 128, L // 2], BF, tag=f"ctxg31{ii}",
                                 name=f"ctxg31{ii}", addr_space="Shared")
                       for ii in range(2)]

            ones_cb = consts.tile([128, 1], BF, tag="ones_cb")
            nc.vector.memset(ones_cb[:], 1.0)
            ones_r = consts.tile([1, 128], F32, tag="ones_r")
            nc.vector.memset(ones_r[:], 1.0)
            ones_rb = consts.tile([1, 128], BF, tag="ones_rb")
            nc.vector.memset(ones_rb[:], 1.0)
            eps2 = consts.tile([128, 1], F32, tag="eps2")
            nc.vector.memset(eps2[:], EPS)
            # ln(sqrt(sc2)): fold of 1/sqrt(HD) for q rows into the exp-form
            # rsqrt (avoids the Sqrt ACT table set entirely)
            lnsc = consts.tile([128, 1], F32, tag="lnsc")
            nc.vector.memset(lnsc[0:64, :], -0.5 * float(np.log(HD)))
            nc.vector.memset(lnsc[64:128, :], 0.0)

            # resident weights + rope tables (host pre-blocked: one
            # contiguous run per partition). Only wq is loaded here; the
            # first x chunk's DMA is emitted next (critical path), then
            # wo/tabs follow.
            wq_sb = wres.tile([128, NDT * 3 * DQ], BF, tag="wq_sb")
            nc.sync.dma_start(wq_sb[:], wqQ[:, :])
            wo_sb = wres.tile([128, NDT * DQ], BF, tag="wo_sb")
            tabs = {}

            def load_rest_of_weights():
                nc.sync.dma_start(wo_sb[:], woQ[:, :])
                for nm, t in tab_ext.items():
                    tt = wres.tile([128, L], BF, tag=nm + "_sb",
                                   name=nm + "_sb")
                    nc.sync.dma_start(tt[:], t[:, :])
                    tabs[nm] = tt

            v_dr = v_dram.rearrange("(a p) q -> p a q", p=128)

            with tc.tile_pool(name="xcp", bufs=2) as xcp, \
                 tc.tile_pool(name="p1sb", bufs=2) as p1sb, \
                 tc.tile_pool(name="qkp", bufs=2, space="PSUM") as qkp, \
                 tc.tile_pool(name="vpp", bufs=1, space="PSUM") as vpp, \
                 tc.tile_pool(name="sp", bufs=3, space="PSUM") as sp, \
                 tc.tile_pool(name="cxp", bufs=2, space="PSUM") as cxp, \
                 tc.tile_pool(name="att", bufs=2) as att, \
                 tc.tile_pool(name="ptp", bufs=3) as ptp, \
                 tc.tile_pool(name="dap", bufs=2) as dap, \
                 tc.tile_pool(name="tlp", bufs=2) as tlp, \
                 tc.tile_pool(name="wsb", bufs=2) as wsb:

                # ---------------- qkv projection units ----------------
                def emit_v_pair(c, xc, tbp):
                    vp = vpp.tile([128, 512], F32, tag="vp", name="vp")
                    for half in range(2):
                        tb_ = tbp * 2 + half
                        hsl = slice(half * 256, (half + 1) * 256)
                        for d in range(NDT):
                            nc.tensor.matmul(
                                vp[:, hsl],
                                xc[:, d * CH + tb_ * 128:
                                   d * CH + (tb_ + 1) * 128],
                                wq_sb[:, d * 3 * DQ + 2 * DQ:
                                      d * 3 * DQ + 3 * DQ],
                                start=(d == 0), stop=(d == NDT - 1))
                    vsb = p1sb.tile([128, 512], BF, tag="vsb")
                    nc.scalar.copy(vsb[:], vp[:])
                    r0 = c * CH + tbp * 256
                    nc.sync.dma_start(v_dram[r0:r0 + 128, :], vsb[:, 0:256])
                    nc.sync.dma_start(v_dram[r0 + 128:r0 + 256, :],
                                      vsb[:, 256:512])

                def emit_ar_ssf(b):
                    # AllReduce sumsq, compute 1/rms (exp/ln form: stays in
                    # the natural_log_exp activation table set)
                    nc.gpsimd.collective_compute(
                        "AllReduce", mybir.AluOpType.add,
                        replica_groups=[list(range(NC))],
                        ins=[ss_in[b].opt()], outs=[ss_out[b].opt()])
                    FW = 2 * L // 128
                    ssf = p1sb.tile([128, FW], F32, tag="ssf")
                    nc.sync.dma_start(
                        ssf[:],
                        ss_out[b].rearrange("a (p f) -> (a p) f", p=64)[:, :])
                    nc.scalar.activation(ssf[:], ssf[:],
                                         mybir.ActivationFunctionType.Ln,
                                         bias=eps2[:], scale=1.0 / D)
                    nc.scalar.activation(ssf[:], ssf[:],
                                         mybir.ActivationFunctionType.Exp,
                                         bias=lnsc[:], scale=-0.5)
                    nc.sync.dma_start(
                        ss_fin[b].rearrange("a (p f) -> (a p) f", p=64)[:, :],
                        ssf[:])
                    ssfb = p1sb.tile([128, FW], BF, tag="ssfb")
                    nc.vector.tensor_copy(ssfb[:], ssf[:])
                    nc.sync.dma_start(
                        ss_finb[b].rearrange("a (p f) -> (a p) f", p=64)[:, :],
                        ssfb[:])

                def qkv_work(b):
                    for lc in range(NCPB):
                        c = b * NCPB + lc
                        csl = slice(c * CH, (c + 1) * CH)
                        tsl = slice(lc * CH, (lc + 1) * CH)
                        xc = xcp.tile([128, NDT * CH], BF, tag="xc")
                        nc.sync.dma_start(xc[:], xQ[c])
                        yield
                        for side in range(2):    # 0=q, 1=k
                            pscs = []
                            sqs_ = []
                            for eo in range(2):  # 0=evens, 1=odds
                                m = side * 2 + eo
                                ps = qkp.tile([128, CH], F32, tag="qk",
                                              name="qk")
                                for d in range(NDT):
                                    nc.tensor.matmul(
                                        ps[:],
                                        wq_sb[:, d * 3 * DQ + m * 128:
                                              d * 3 * DQ + (m + 1) * 128],
                                        xc[:, d * CH:(d + 1) * CH],
                                        start=(d == 0), stop=(d == NDT - 1))
                                sq = p1sb.tile([128, CH], BF, tag="sq",
                                               name="sq", bufs=3)
                                nc.scalar.square(sq[:], ps[:])
                                sqs_.append(sq)
                                psc = p1sb.tile([128, CH], BF, tag="psc",
                                                name="psc", bufs=3)
                                nc.scalar.copy(psc[:], ps[:])
                                pscs.append(psc)
                                yield
                            ssp = sp.tile([128, CH], F32, tag="sp", name="ssp")
                            for eo in range(2):
                                nc.tensor.matmul(ssp[0:1, :], ones_cb[:],
                                                 sqs_[eo][:],
                                                 start=(eo == 0),
                                                 stop=(eo == 1))
                            ssr = p1sb.tile([1, CH], F32, tag="ssr", bufs=1)
                            nc.scalar.copy(ssr[:], ssp[0:1, :])
                            nc.sync.dma_start(
                                ss_in[b][side:side + 1, tsl], ssr[:])
                            # rope (norm weights folded into tables on host)
                            psE, psO = pscs
                            pre = "tq" if side == 0 else "tk"
                            ta = p1sb.tile([128, CH], BF, tag="ta", bufs=1)
                            nc.vector.tensor_mul(ta[:], psE[:],
                                                 tabs[pre + "ce"][:, tsl])
                            tb = p1sb.tile([128, CH], BF, tag="tb", bufs=1)
                            nc.vector.tensor_mul(tb[:], psO[:],
                                                 tabs[pre + "so"][:, tsl])
                            roE = p1sb.tile([128, CH], BF, tag="roE")
                            nc.vector.tensor_sub(roE[:], ta[:], tb[:])
                            tc_ = p1sb.tile([128, CH], BF, tag="tc_", bufs=1)
                            nc.vector.tensor_mul(tc_[:], psO[:],
                                                 tabs[pre + "co"][:, tsl])
                            td = p1sb.tile([128, CH], BF, tag="td", bufs=1)
                            nc.vector.tensor_mul(td[:], psE[:],
                                                 tabs[pre + "se"][:, tsl])
                            roO = p1sb.tile([128, CH], BF, tag="roO")
                            nc.vector.tensor_add(roO[:], tc_[:], td[:])
                            # q/k stored in PSUM row order: [evens h0, evens
                            # h1, odds h0, odds h1]
                            dst = q_dram if side == 0 else k_dram
                            nc.sync.dma_start(dst[0:128, csl], roE[:])
                            nc.sync.dma_start(dst[128:256, csl], roO[:])
                            yield
                        if lc < NCPB - 1:
                            for tbp in range(2):
                                emit_v_pair(c, xc, tbp)
                                yield
                        else:
                            # last chunk: AllReduce first, v work covers its
                            # latency
                            emit_ar_ssf(b)
                            yield
                            for tbp in range(2):
                                emit_v_pair(c, xc, tbp)
                                yield

                QKV_UNITS = NCPB * 9 + 1   # 37

                # ---------------- wo projection units ----------------
                def wo_work(b):
                    for i in range(L // 512):
                        cg = wsb.tile([128, NDT * 512], BF, tag="cg")
                        cgr = cg.rearrange("p (a t) -> p a t", a=NDT)
                        isl = slice(i * 512, (i + 1) * 512)
                        nc.sync.dma_start(
                            cgr[:, 0:8, :],
                            ctx_gh[b][0].rearrange("(a p) t -> p a t",
                                                   p=128)[:, :, isl])
                        if b == B - 1:
                            half = i // 2
                            co = i * 512 - half * 1024
                            nc.sync.dma_start(
                                cgr[:, 8:16, :],
                                ctx_g31[half].rearrange(
                                    "(a p) t -> p a t",
                                    p=128)[:, :, co:co + 512])
                        else:
                            nc.sync.dma_start(
                                cgr[:, 8:16, :],
                                ctx_gh[b][1].rearrange("(a p) t -> p a t",
                                                       p=128)[:, :, isl])
                        yield
                        for m in range(DQ // 128):
                            op = sp.tile([128, 512], F32, tag="sp", name="wop")
                            for d in range(NDT):
                                nc.tensor.matmul(
                                    op[:],
                                    wo_sb[:, d * DQ + m * 128:
                                          d * DQ + (m + 1) * 128],
                                    cg[:, d * 512:(d + 1) * 512],
                                    start=(d == 0), stop=(d == NDT - 1))
                            osb = p1sb.tile([128, 512], F32, tag="osb")
                            nc.scalar.copy(osb[:], op[:])
                            nc.sync.dma_start(
                                outT[m * 128:(m + 1) * 128,
                                     b * L + i * 512:b * L + (i + 1) * 512],
                                osb[:])
                            yield

                WO_UNITS = (L // 512) * 3  # 12

                # ---------------- attention spine ----------------
                def attn_spine(b):
                    bsl = slice(b * L, (b + 1) * L)
                    sqs = att.tile([1, L], BF, tag="sqs", bufs=1)
                    nc.sync.dma_start(sqs[:], ss_finb[b][0:1, :])
                    skc = att.tile([128, NJ], F32, tag="skc")
                    nc.sync.dma_start(
                        skc[:],
                        ss_fin[b][1:2, :].rearrange("a (j p) -> (a p) j",
                                                    p=128))
                    yield
                    for hl in range(2):
                        qn = att.tile([128, L], BF, tag="qn")
                        nc.sync.dma_start(
                            qn[0:64, :], q_dram[hl * 64:(hl + 1) * 64, bsl])
                        nc.sync.dma_start(
                            qn[64:128, :],
                            q_dram[128 + hl * 64:128 + (hl + 1) * 64, bsl])
                        kn = att.tile([128, L], BF, tag="kn")
                        nc.sync.dma_start(
                            kn[0:64, :], k_dram[hl * 64:(hl + 1) * 64, bsl])
                        nc.sync.dma_start(
                            kn[64:128, :],
                            k_dram[128 + hl * 64:128 + (hl + 1) * 64, bsl])
                        vh = att.tile([128, NJ * 128], BF, tag="vh")
                        nc.sync.dma_start(
                            vh.rearrange("p (a q) -> p a q", a=NJ),
                            v_dr[:, b * NJ:(b + 1) * NJ,
                                 hl * 128:(hl + 1) * 128])
                        for ii in range(L // 512):
                            isl = slice(ii * 512, (ii + 1) * 512)
                            bp = sp.tile([128, 512], F32, tag="sp", name="bp")
                            nc.tensor.matmul(
                                bp[:], ones_rb[:], sqs[0:1, isl],
                                start=True, stop=True)
                            nc.vector.tensor_mul(qn[:, isl], qn[:, isl],
                                                 bp[:])
                        yield
                        for g in range(L // (2 * IW)):
                            ics = (2 * g, 2 * g + 1)
                            cps = {}
                            dacc = {}
                            for ic in ics:
                                cps[ic] = cxp.tile([128, IW], F32, tag="cx",
                                                   name=f"cx{ic % 2}")
                                dacc[ic] = [dap.tile([128, IW], BF,
                                                     tag=f"da{ic % 2}{e}",
                                                     name=f"da{ic % 2}{e}")
                                            for e in range(2)]
                            for j in range(NJ):
                                for ic in ics:
                                    sps = sp.tile([128, IW], F32, tag="sp",
                                                  name="sps")
                                    nc.tensor.matmul(
                                        sps[:],
                                        kn[:, j * 128:(j + 1) * 128],
                                        qn[:, ic * IW:(ic + 1) * IW],
                                        start=True, stop=True)
                                    pt = ptp.tile([128, IW], BF, tag="pt",
                                                  name="pt")
                                    nc.scalar.activation(
                                        pt[:], sps[:],
                                        mybir.ActivationFunctionType.Exp,
                                        scale=skc[:, j:j + 1])
                                    da = dacc[ic][j % 2]
                                    if j < 2:
                                        nc.vector.tensor_copy(da[:], pt[:])
                                    else:
                                        nc.vector.tensor_add(da[:], da[:],
                                                             pt[:])
                                    nc.tensor.matmul(
                                        cps[ic][:],
                                        vh[:, j * 128:(j + 1) * 128],
                                        pt[:],
                                        start=(j == 0), stop=(j == NJ - 1))
                                yield
                            for ic in ics:
                                df = tlp.tile([128, IW], BF, tag="df")
                                nc.vector.tensor_add(df[:], dacc[ic][0][:],
                                                     dacc[ic][1][:])
                                dps = sp.tile([128, 512], F32, tag="sp",
                                              name="dps")
                                nc.tensor.matmul(dps[0:1, :], ones_cb[:],
                                                 df[:], start=True, stop=True)
                                rrow = tlp.tile([1, 512], F32, tag="rrow")
                                nc.vector.reciprocal_approx_fast(
                                    rrow[:], dps[0:1, :])
                                rrb = tlp.tile([1, 512], BF, tag="rrb")
                                nc.vector.tensor_copy(rrb[:], rrow[:])
                                rbp = sp.tile([128, 512], F32, tag="sp",
                                              name="rbp")
                                nc.tensor.matmul(rbp[:], ones_rb[:], rrb[:],
                                                 start=True, stop=True)
                                rbs = tlp.tile([128, 512], BF, tag="rbs")
                                nc.scalar.copy(rbs[:], rbp[:])
                                csb = tlp.tile([128, 512], BF, tag="csb")
                                nc.vector.tensor_mul(csb[:], cps[ic][:],
                                                     rbs[:])
                                if b == B - 1 and hl == 1:
                                    nc.sync.dma_start(
                                        ctx_b31[g][:, (ic - 2 * g) * IW:
                                                   (ic - 2 * g + 1) * IW],
                                        csb[:])
                                else:
                                    nc.sync.dma_start(
                                        ctx_bh[b][hl][:,
                                                      ic * IW:(ic + 1) * IW],
                                        csb[:])
                                yield
                            if b == B - 1 and hl == 1:
                                nc.gpsimd.collective_compute(
                                    "AllGather", mybir.AluOpType.bypass,
                                    replica_groups=[list(range(NC))],
                                    ins=[ctx_b31[g].opt()],
                                    outs=[ctx_g31[g].opt()])
                                yield
                        if not (b == B - 1 and hl == 1):
                            nc.gpsimd.collective_compute(
                                "AllGather", mybir.AluOpType.bypass,
                                replica_groups=[list(range(NC))],
                                ins=[ctx_bh[b][hl].opt()],
                                outs=[ctx_gh[b][hl].opt()])
                            yield

                # spine yields per batch: 1 + 2*(1 + 2*(16+2) + 1) = 77
                SPINE_SLOTS = 77

                qkv_gens = [qkv_work(b) for b in range(B)]
                wo_gens = [wo_work(b) for b in range(B)]

                # prologue: xc0 DMA first (critical path), then remaining
                # resident weights, then project batch 0 alone
                next(qkv_gens[0])
                load_rest_of_weights()
                f0 = Feeder(qkv_gens[0], 0, 0, 0)
                f0.gen = qkv_gens[0]
                f0.done = False
                f0.drain()

                for b in range(B):
                    feeders = []
                    if b + 1 < B:
                        feeders.append(
                            Feeder(qkv_gens[b + 1], QKV_UNITS, 2, 56))
                    if b >= 1:
                        feeders.append(
                            Feeder(wo_gens[b - 1], WO_UNITS, 30, SPINE_SLOTS - 2))
                    slot = 0
                    for _ in attn_spine(b):
                        slot += 1
                        for f in feeders:
                            f.step(slot)
                    for f in feeders:
                        f.drain()

                fN = Feeder(wo_gens[B - 1], 0, 0, 0)
                fN.gen = wo_gens[B - 1]
                fN.done = False
                fN.drain()

    nc.compile()
    return nc


def _prep_inputs(x_BLD, freqs, wqkv, wo, q_norm_w, k_norm_w):
    """Host-side sharding/layout. Returns in_maps (list of 8 dicts)."""
    x = np.asarray(x_BLD, np.float32)
    freqs = np.asarray(freqs, np.float32)
    wqkv = np.asarray(wqkv, np.float32)
    wo = np.asarray(wo, np.float32)
    qw = np.asarray(q_norm_w, np.float32)
    kw = np.asarray(k_norm_w, np.float32)

    # xQ[c] = chunk c's SBUF image: [128, a*CH + t] = x[c*CH+t, a*128+p]
    xQ = np.ascontiguousarray(
        x.reshape(T, D).astype(BF_NP)
        .reshape(B * NCPB, CH, NDT, 128).transpose(0, 3, 2, 1)
        .reshape(B * NCPB, 128, NDT * CH))
    sinT = np.ascontiguousarray(freqs[0].T)  # [D/2, L]
    cosT = np.ascontiguousarray(freqs[1].T)

    evens = 2 * np.arange(64)
    odds = evens + 1
    # ctx order after split AllGather: all even heads, then all odd heads
    woperm = np.concatenate(
        [h * HD + np.arange(HD) for h in range(0, H, 2)]
        + [h * HD + np.arange(HD) for h in range(1, H, 2)])

    in_maps = []
    for r in range(NC):
        heads = [HPC * r + hl for hl in range(HPC)]
        # q/k row order: [h0 evens, h1 evens, h0 odds, h1 odds]
        qrows = np.concatenate([h * HD + evens for h in heads]
                               + [h * HD + odds for h in heads])
        rows = np.concatenate([qrows, D + qrows, 2 * D + DQ * r + np.arange(DQ)])
        wqkvT = wqkv[rows, :].T.astype(BF_NP)          # [D, 3DQ]
        wqQ = np.ascontiguousarray(
            wqkvT.reshape(NDT, 128, 3 * DQ).transpose(1, 0, 2)
            .reshape(128, NDT * 3 * DQ))
        woT = wo[DQ * r:DQ * (r + 1), :][:, woperm].T.astype(BF_NP)
        woQ = np.ascontiguousarray(
            woT.reshape(NDT, 128, DQ).transpose(1, 0, 2)
            .reshape(128, NDT * DQ))

        tabs = {p + sfx: np.empty((HD, L), np.float32)
                for p in ("tq", "tk") for sfx in ("ce", "so", "co", "se")}
        for hl, h in enumerate(heads):
            rsl = slice(hl * 64, (hl + 1) * 64)
            cosP = cosT[h * 64:(h + 1) * 64]
            sinP = sinT[h * 64:(h + 1) * 64]
            for w, p in ((qw, "tq"), (kw, "tk")):
                w_e = w[h * HD + evens][:, None]
                w_o = w[h * HD + odds][:, None]
                tabs[p + "ce"][rsl] = w_e * cosP
                tabs[p + "so"][rsl] = w_o * sinP
                tabs[p + "co"][rsl] = w_o * cosP
                tabs[p + "se"][rsl] = w_e * sinP

        in_maps.append({
            "xQ": xQ,
            "wqQ": wqQ,
            "woQ": woQ,
            **{k: v.astype(BF_NP) for k, v in tabs.items()},
        })
    return in_maps


def _assemble(results):
    outT = np.empty((D, T), np.float32)
    for r in range(NC):
        outT[DQ * r:DQ * (r + 1)] = results[r]["outT"]
    return np.ascontiguousarray(outT.T).reshape(B, L, D)


def _install_ntff_hook():
    """The agent image's antenv lacks axon_hooks; provide the documented shim
    so run_bass_kernel_spmd(trace=True) can NTFF-profile via libaxon_pjrt."""
    try:
        import antenv.axon_hooks  # noqa: F401
        return
    except ImportError:
        pass
    import types
    hookf = None
    try:
        from trn_agent_boot.trn_boot import _ntff_profile_via_ctypes
        hookf = _ntff_profile_via_ctypes("/opt/axon/libaxon_pjrt.so")
    except Exception:
        pass
    mod = types.ModuleType("antenv.axon_hooks")
    state = {"h": hookf}
    mod.set_axon_ntff_profile_hook = lambda h: state.__setitem__("h", h)
    mod.get_axon_ntff_profile_hook = lambda: state["h"]
    sys.modules["antenv.axon_hooks"] = mod
    import antenv
    antenv.axon_hooks = mod


def kernel(x_BLD, freqs, wqkv, wo, q_norm_w, k_norm_w, _trace=False):
    from concourse.bass_utils import run_bass_kernel_spmd
    if _trace:
        _install_ntff_hook()
    if "nc" not in _CACHE:
        _CACHE["nc"] = build_nc()
    nc = _CACHE["nc"]
    in_maps = _prep_inputs(x_BLD, freqs, wqkv, wo, q_norm_w, k_norm_w)
    res = run_bass_kernel_spmd(nc, in_maps, core_ids=list(range(NC)),
                               trace=_trace)
    out = _assemble(res.results)
    if _trace:
        return out, res
    return out


# revision 40
# speedup vs baseline: 1.0229x; 1.0191x over previous
"""Distributed Trainium2 kernel for nn_Attention_79207786873592.

Full attention block: qkv projection -> RMSNorm(q,k) -> RoPE -> SDPA -> wo.
B=4, L=2048, D=2048, H=16 heads, head_dim=128, fp32 I/O (bf16 compute).

Sharding: 8-way tensor-parallel over heads (2 heads/core), as in v1.

v2: software-pipelined fusion. The attention phase is ACT-paced (exp is
~780ns per [128,512] tile while the two PE matmuls of the same iteration
only need ~430ns), so the PE-bound qkv projection of batch b+1 and the
wo projection of batch b-1 are emitted interleaved into attention(b)'s
j-loop. Generators yield between work units; a static scheduler injects
qkv/wo units at attention yield points so every engine stays busy.

Other changes vs v1:
  - v is computed directly in [token, dout] layout (x-block stationary,
    wqkv-v columns moving), eliminating the PE transposes.
  - softmax tails use reciprocal_approx_fast (DVE custom op, ~670ns) in
    place of RECIPROCAL (~4us).
  - AllGather is split per (batch, head-pair) halves for earlier overlap;
    wo weights are host-permuted to [even heads, odd heads] ctx order.
  - PSUM banks: qk-proj 2, v-proj 1, scores/misc 3 (shared ring), ctx 2.
"""

import sys

sys.path.insert(0, "/opt/trn_rl_repo")

import numpy as np
import ml_dtypes

import concourse.bass as bass
import concourse.tile as tile
import concourse.mybir as mybir
from concourse import bacc

B, L, D, H = 4, 2048, 2048, 16
HD = D // H              # 128
NC = 8                   # cores
HPC = H // NC            # 2 heads per core
DQ = HPC * HD            # 256 rows of q/k/v per core
T = B * L                # 8192 tokens
EPS = 1e-5
CH = 512                 # token chunk
NCPB = L // CH           # 4 chunks per batch
NDT = D // 128           # 16 contraction tiles
NJ = L // 128            # 16 k-tiles per batch
IW = 512                 # q-position chunk width
BF = mybir.dt.bfloat16
F32 = mybir.dt.float32
BF_NP = ml_dtypes.bfloat16

_CACHE = {}


class Feeder:
    """Emits units of a generator at scheduled spine-yield slots."""

    def __init__(self, gen, n_units, first, last):
        self.gen = gen
        self.done = False
        if n_units > 0:
            span = max(1, last - first)
            self.slots = [first + (k * span) // max(1, n_units - 1)
                          if n_units > 1 else first
                          for k in range(n_units)]
        else:
            self.slots = []
        self.idx = 0

    def step(self, slot):
        while (not self.done and self.idx < len(self.slots)
               and self.slots[self.idx] <= slot):
            try:
                next(self.gen)
                self.idx += 1
            except StopIteration:
                self.done = True

    def drain(self):
        while not self.done:
            try:
                next(self.gen)
            except StopIteration:
                self.done = True


def build_nc():
    nc = bacc.Bacc("TRN2", target_bir_lowering=False, debug=False, num_devices=NC)

    # pre-blocked on host: xQ[c] is exactly one chunk's SBUF image
    # (16KB contiguous per partition), wqQ/woQ are the SBUF weight images
    xQ = nc.dram_tensor("xQ", [B * NCPB, 128, NDT * CH], BF,
                        kind="ExternalInput").ap()
    wqQ = nc.dram_tensor("wqQ", [128, NDT * 3 * DQ], BF,
                         kind="ExternalInput").ap()
    woQ = nc.dram_tensor("woQ", [128, NDT * DQ], BF,
                         kind="ExternalInput").ap()
    TAB_NAMES = [p + sfx for p in ("tq", "tk")
                 for sfx in ("ce", "so", "co", "se")]
    tab_ext = {nm: nc.dram_tensor(nm, [HD, L], BF, kind="ExternalInput").ap()
               for nm in TAB_NAMES}
    outT = nc.dram_tensor("outT", [DQ, T], F32, kind="ExternalOutput").ap()

    with tile.TileContext(nc) as tc:
        with tc.tile_pool(name="dram", bufs=1, space="DRAM") as dram, \
             tc.tile_pool(name="consts", bufs=1) as consts, \
             tc.tile_pool(name="wres", bufs=1) as wres:

            q_dram = dram.tile([DQ, T], BF, tag="q_dram")
            k_dram = dram.tile([DQ, T], BF, tag="k_dram")
            v_dram = dram.tile([T, DQ], BF, tag="v_dram")
            ss_in = [dram.tile([2, L], F32, tag=f"ss_in{b}", name=f"ss_in{b}")
                     for b in range(B)]
            ss_out = [dram.tile([2, L], F32, tag=f"ss_out{b}", name=f"ss_out{b}",
                                addr_space="Shared") for b in range(B)]
            ss_fin = [dram.tile([2, L], F32, tag=f"ss_fin{b}", name=f"ss_fin{b}")
                      for b in range(B)]
            ss_finb = [dram.tile([2, L], BF, tag=f"ss_finb{b}",
                                 name=f"ss_finb{b}") for b in range(B)]
            ctx_bh = [[dram.tile([128, L], BF, tag=f"ctxb{b}h{hl}",
                                 name=f"ctxb{b}h{hl}") for hl in range(2)]
                      for b in range(B)]
            ctx_gh = [[dram.tile([NC * 128, L], BF, tag=f"ctxg{b}h{hl}",
                                 name=f"ctxg{b}h{hl}", addr_space="Shared")
                       for hl in range(2)] for b in range(B)]
            # batch 3 / hl 1: column-split halves so the final AllGather
            # exposes only half the transfer after the last attention group
            ctx_b31 = [dram.tile([128, L // 2], BF, tag=f"ctxb31{ii}",
                                 name=f"ctxb31{ii}") for ii in range(2)]
            ctx_g31 = [dram.tile([NC * 128, L // 2], BF, tag=f"ctxg31{ii}",
                                 name=f"ctxg31{ii}", addr_space="Shared")
                       for ii in range(2)]

            ones_cb = consts.tile([128, 1], BF, tag="ones_cb")
            nc.vector.memset(ones_cb[:], 1.0)
            ones_r = consts.tile([1, 128], F32, tag="ones_r")
            nc.vector.memset(ones_r[:], 1.0)
            ones_rb = consts.tile([1, 128], BF, tag="ones_rb")
            nc.vector.memset(ones_rb[:], 1.0)
            eps2 = consts.tile([128, 1], F32, tag="eps2")
            nc.vector.memset(eps2[:], EPS)
            # ln(sqrt(sc2)): fold of 1/sqrt(HD) for q rows into the exp-form
            # rsqrt (avoids the Sqrt ACT table set entirely)
            lnsc = consts.tile([128, 1], F32, tag="lnsc")
            nc.vector.memset(lnsc[0:64, :], -0.5 * float(np.log(HD)))
            nc.vector.memset(lnsc[64:128, :], 0.0)

            # resident weights + rope tables (host pre-blocked: one
            # contiguous run per partition). Only wq is loaded here; the
            # first x chunk's DMA is emitted next (critical path), then
            # wo/tabs follow.
            wq_sb = wres.tile([128, NDT * 3 * DQ], BF, tag="wq_sb")
            nc.sync.dma_start(wq_sb[:], wqQ[:, :])
            wo_sb = wres.tile([128, NDT * DQ], BF, tag="wo_sb")
            tabs = {}

            def load_rest_of_weights():
                nc.sync.dma_start(wo_sb[:], woQ[:, :])
                for nm, t in tab_ext.items():
                    tt = wres.tile([128, L], BF, tag=nm + "_sb",
                                   name=nm + "_sb")
                    nc.sync.dma_start(tt[:], t[:, :])
                    tabs[nm] = tt

            v_dr = v_dram.rearrange("(a p) q -> p a q", p=128)

            with tc.tile_pool(name="xcp", bufs=2) as xcp, \
                 tc.tile_pool(name="p1sb", bufs=2) as p1sb, \
                 tc.tile_pool(name="qkp", bufs=2, space="PSUM") as qkp, \
                 tc.tile_pool(name="vpp", bufs=1, space="PSUM") as vpp, \
                 tc.tile_pool(name="sp", bufs=3, space="PSUM") as sp, \
                 tc.tile_pool(name="cxp", bufs=2, space="PSUM") as cxp, \
                 tc.tile_pool(name="att", bufs=2) as att, \
                 tc.tile_pool(name="ptp", bufs=3) as ptp, \
                 tc.tile_pool(name="dap", bufs=2) as dap, \
                 tc.tile_pool(name="tlp", bufs=2) as tlp, \
                 tc.tile_pool(name="wsb", bufs=2) as wsb:

                # ---------------- qkv projection units ----------------
                def emit_v_pair(c, xc, tbp):
                    vp = vpp.tile([128, 512], F32, tag="vp", name="vp")
                    for half in range(2):
                        tb_ = tbp * 2 + half
                        hsl = slice(half * 256, (half + 1) * 256)
                        for d in range(NDT):
                            nc.tensor.matmul(
                                vp[:, hsl],
                                xc[:, d * CH + tb_ * 128:
                                   d * CH + (tb_ + 1) * 128],
                                wq_sb[:, d * 3 * DQ + 2 * DQ:
                                      d * 3 * DQ + 3 * DQ],
                                start=(d == 0), stop=(d == NDT - 1))
                    vsb = p1sb.tile([128, 512], BF, tag="vsb")
                    nc.scalar.copy(vsb[:], vp[:])
                    r0 = c * CH + tbp * 256
                    nc.sync.dma_start(v_dram[r0:r0 + 128, :], vsb[:, 0:256])
                    nc.sync.dma_start(v_dram[r0 + 128:r0 + 256, :],
                                      vsb[:, 256:512])

                def emit_ar_ssf(b):
                    # AllReduce sumsq, compute 1/rms (exp/ln form: stays in
                    # the natural_log_exp activation table set)
                    nc.gpsimd.collective_compute(
                        "AllReduce", mybir.AluOpType.add,
                        replica_groups=[list(range(NC))],
                        ins=[ss_in[b].opt()], outs=[ss_out[b].opt()])
                    FW = 2 * L // 128
                    ssf = p1sb.tile([128, FW], F32, tag="ssf")
                    nc.sync.dma_start(
                        ssf[:],
                        ss_out[b].rearrange("a (p f) -> (a p) f", p=64)[:, :])
                    nc.scalar.activation(ssf[:], ssf[:],
                                         mybir.ActivationFunctionType.Ln,
                                         bias=eps2[:], scale=1.0 / D)
                    nc.scalar.activation(ssf[:], ssf[:],
                                         mybir.ActivationFunctionType.Exp,
                                         bias=lnsc[:], scale=-0.5)
                    nc.sync.dma_start(
                        ss_fin[b].rearrange("a (p f) -> (a p) f", p=64)[:, :],
                        ssf[:])
                    ssfb = p1sb.tile([128, FW], BF, tag="ssfb")
                    nc.vector.tensor_copy(ssfb[:], ssf[:])
                    nc.sync.dma_start(
                        ss_finb[b].rearrange("a (p f) -> (a p) f", p=64)[:, :],
                        ssfb[:])

                def qkv_work(b):
                    for lc in range(NCPB):
                        c = b * NCPB + lc
                        csl = slice(c * CH, (c + 1) * CH)
                        tsl = slice(lc * CH, (lc + 1) * CH)
                        xc = xcp.tile([128, NDT * CH], BF, tag="xc")
                        nc.sync.dma_start(xc[:], xQ[c])
                        yield
                        for side in range(2):    # 0=q, 1=k
                            pscs = []
                            sqs_ = []
                            for eo in range(2):  # 0=evens, 1=odds
                                m = side * 2 + eo
                                ps = qkp.tile([128, CH], F32, tag="qk",
                                              name="qk")
                                for d in range(NDT):
                                    nc.tensor.matmul(
                                        ps[:],
                                        wq_sb[:, d * 3 * DQ + m * 128:
                                              d * 3 * DQ + (m + 1) * 128],
                                        xc[:, d * CH:(d + 1) * CH],
                                        start=(d == 0), stop=(d == NDT - 1))
                                sq = p1sb.tile([128, CH], BF, tag="sq",
                                               name="sq", bufs=3)
                                nc.scalar.square(sq[:], ps[:])
                                sqs_.append(sq)
                                psc = p1sb.tile([128, CH], BF, tag="psc",
                                                name="psc", bufs=3)
                                nc.scalar.copy(psc[:], ps[:])
                                pscs.append(psc)
                                yield
                            ssp = sp.tile([128, CH], F32, tag="sp", name="ssp")
                            for eo in range(2):
                                nc.tensor.matmul(ssp[0:1, :], ones_cb[:],
                                                 sqs_[eo][:],
                                                 start=(eo == 0),
                                                 stop=(eo == 1))
                            ssr = p1sb.tile([1, CH], F32, tag="ssr", bufs=1)
                            nc.scalar.copy(ssr[:], ssp[0:1, :])
                            nc.sync.dma_start(
                                ss_in[b][side:side + 1, tsl], ssr[:])
                            # rope (norm weights folded into tables on host)
                            psE, psO = pscs
                            pre = "tq" if side == 0 else "tk"
                            ta = p1sb.tile([128, CH], BF, tag="ta", bufs=1)
                            nc.vector.tensor_mul(ta[:], psE[:],
                                                 tabs[pre + "ce"][:, tsl])
                            tb = p1sb.tile([128, CH], BF, tag="tb", bufs=1)
                            nc.vector.tensor_mul(tb[:], psO[:],
                                                 tabs[pre + "so"][:, tsl])
                            roE = p1sb.tile([128, CH], BF, tag="roE")
                            nc.vector.tensor_sub(roE[:], ta[:], tb[:])
                            tc_ = p1sb.tile([128, CH], BF, tag="tc_", bufs=1)
                            nc.vector.tensor_mul(tc_[:], psO[:],
                                                 tabs[pre + "co"][:, tsl])
                            td = p1sb.tile([128, CH], BF, tag="td", bufs=1)
                            nc.vector.tensor_mul(td[:], psE[:],
                                                 tabs[pre + "se"][:, tsl])
                            roO = p1sb.tile([128, CH], BF, tag="roO")
                            nc.vector.tensor_add(roO[:], tc_[:], td[:])
                            dst = q_dram if side == 0 else k_dram
                            for hl in range(HPC):
                                nc.sync.dma_start(
                                    dst[hl * 128:hl * 128 + 64, csl],
                                    roE[hl * 64:(hl + 1) * 64, :])
                                nc.sync.dma_start(
                                    dst[hl * 128 + 64:(hl + 1) * 128, csl],
                                    roO[hl * 64:(hl + 1) * 64, :])
                            yield
                        if lc < NCPB - 1:
                            for tbp in range(2):
                                emit_v_pair(c, xc, tbp)
                                yield
                        else:
                            # last chunk: AllReduce first, v work covers its
                            # latency
                            emit_ar_ssf(b)
                            yield
                            for tbp in range(2):
                                emit_v_pair(c, xc, tbp)
                                yield

                QKV_UNITS = NCPB * 9 + 1   # 37

                # ---------------- wo projection units ----------------
                def wo_work(b):
                    for i in range(L // 512):
                        cg = wsb.tile([128, NDT * 512], BF, tag="cg")
                        cgr = cg.rearrange("p (a t) -> p a t", a=NDT)
                        isl = slice(i * 512, (i + 1) * 512)
                        nc.sync.dma_start(
                            cgr[:, 0:8, :],
                            ctx_gh[b][0].rearrange("(a p) t -> p a t",
                                                   p=128)[:, :, isl])
                        if b == B - 1:
                            half = i // 2
                            co = i * 512 - half * 1024
                            nc.sync.dma_start(
                                cgr[:, 8:16, :],
                                ctx_g31[half].rearrange(
                                    "(a p) t -> p a t",
                                    p=128)[:, :, co:co + 512])
                        else:
                            nc.sync.dma_start(
                                cgr[:, 8:16, :],
                                ctx_gh[b][1].rearrange("(a p) t -> p a t",
                                                       p=128)[:, :, isl])
                        yield
                        for m in range(DQ // 128):
                            op = sp.tile([128, 512], F32, tag="sp", name="wop")
                            for d in range(NDT):
                                nc.tensor.matmul(
                                    op[:],
                                    wo_sb[:, d * DQ + m * 128:
                                          d * DQ + (m + 1) * 128],
                                    cg[:, d * 512:(d + 1) * 512],
                                    start=(d == 0), stop=(d == NDT - 1))
                            osb = p1sb.tile([128, 512], F32, tag="osb")
                            nc.scalar.copy(osb[:], op[:])
                            nc.sync.dma_start(
                                outT[m * 128:(m + 1) * 128,
                                     b * L + i * 512:b * L + (i + 1) * 512],
                                osb[:])
                            yield

                WO_UNITS = (L // 512) * 3  # 12

                # ---------------- attention spine ----------------
                def attn_spine(b):
                    bsl = slice(b * L, (b + 1) * L)
                    sqs = att.tile([1, L], BF, tag="sqs", bufs=1)
                    nc.sync.dma_start(sqs[:], ss_finb[b][0:1, :])
                    skc = att.tile([128, NJ], F32, tag="skc")
                    nc.sync.dma_start(
                        skc[:],
                        ss_fin[b][1:2, :].rearrange("a (j p) -> (a p) j",
                                                    p=128))
                    yield
                    for hl in range(2):
                        qn = att.tile([128, L], BF, tag="qn")
                        nc.sync.dma_start(qn[:],
                                          q_dram[hl * 128:(hl + 1) * 128, bsl])
                        kn = att.tile([128, L], BF, tag="kn")
                        nc.sync.dma_start(kn[:],
                                          k_dram[hl * 128:(hl + 1) * 128, bsl])
                        vh = att.tile([128, NJ * 128], BF, tag="vh")
                        nc.sync.dma_start(
                            vh.rearrange("p (a q) -> p a q", a=NJ),
                            v_dr[:, b * NJ:(b + 1) * NJ,
                                 hl * 128:(hl + 1) * 128])
                        for ii in range(L // 512):
                            isl = slice(ii * 512, (ii + 1) * 512)
                            bp = sp.tile([128, 512], F32, tag="sp", name="bp")
                            nc.tensor.matmul(
                                bp[:], ones_rb[:], sqs[0:1, isl],
                                start=True, stop=True)
                            nc.vector.tensor_mul(qn[:, isl], qn[:, isl],
                                                 bp[:])
                        yield
                        for g in range(L // (2 * IW)):
                            ics = (2 * g, 2 * g + 1)
                            cps = {}
                            dacc = {}
                            for ic in ics:
                                cps[ic] = cxp.tile([128, IW], F32, tag="cx",
                                                   name=f"cx{ic % 2}")
                                dacc[ic] = [dap.tile([128, IW], BF,
                                                     tag=f"da{ic % 2}{e}",
                                                     name=f"da{ic % 2}{e}")
                                            for e in range(2)]
                            for j in range(NJ):
                                for ic in ics:
                                    sps = sp.tile([128, IW], F32, tag="sp",
                                                  name="sps")
                                    nc.tensor.matmul(
                                        sps[:],
                                        kn[:, j * 128:(j + 1) * 128],
                                        qn[:, ic * IW:(ic + 1) * IW],
                                        start=True, stop=True)
                                    pt = ptp.tile([128, IW], BF, tag="pt",
                                                  name="pt")
                                    nc.scalar.activation(
                                        pt[:], sps[:],
                                        mybir.ActivationFunctionType.Exp,
                                        scale=skc[:, j:j + 1])
                                    da = dacc[ic][j % 2]
                                    if j < 2:
                                        nc.vector.tensor_copy(da[:], pt[:])
                                    else:
                                        nc.vector.tensor_add(da[:], da[:],
                                                             pt[:])
                                    nc.tensor.matmul(
                                        cps[ic][:],
                                        vh[:, j * 128:(j + 1) * 128],
                                        pt[:],
                                        start=(j == 0), stop=(j == NJ - 1))
                                yield
                            for ic in ics:
                                df = tlp.tile([128, IW], BF, tag="df")
                                nc.vector.tensor_add(df[:], dacc[ic][0][:],
                                                     dacc[ic][1][:])
                                dps = sp.tile([128, 512], F32, tag="sp",
                                              name="dps")
                                nc.tensor.matmul(dps[0:1, :], ones_cb[:],
                                                 df[:], start=True, stop=True)
                                rrow = tlp.tile([1, 512], F32, tag="rrow")
                                nc.vector.reciprocal_approx_fast(
                                    rrow[:], dps[0:1, :])
                                rrb = tlp.tile([1, 512], BF, tag="rrb")
                                nc.vector.tensor_copy(rrb[:], rrow[:])
                                rbp = sp.tile([128, 512], F32, tag="sp",
                                              name="rbp")
                                nc.tensor.matmul(rbp[:], ones_rb[:], rrb[:],
                                                 start=True, stop=True)
                                rbs = tlp.tile([128, 512], BF, tag="rbs")
                                nc.scalar.copy(rbs[:], rbp[:])
                                csb = tlp.tile([128, 512], BF, tag="csb")
                                nc.vector.tensor_mul(csb[:], cps[ic][:],
                                                     rbs[:])
                                if b == B - 1 and hl == 1:
                                    nc.sync.dma_start(
                                        ctx_b31[g][:, (ic - 2 * g) * IW:
                                                   (ic - 2 * g + 1) * IW],
                                        csb[:])
                                else:
                                    nc.sync.dma_start(
                                        ctx_bh[b][hl][:,
                                                      ic * IW:(ic + 1) * IW],
                                        csb[:])
                                yield
                            if b == B - 1 and hl == 1:
                                nc.gpsimd.collective_compute(
                                    "AllGather", mybir.AluOpType.bypass,
                                    replica_groups=[list(range(NC))],
                                    ins=[ctx_b31[g].opt()],
                                    outs=[ctx_g31[g].opt()])
                                yield
                        if not (b == B - 1 and hl == 1):
                            nc.gpsimd.collective_compute(
                                "AllGather", mybir.AluOpType.bypass,
                                replica_groups=[list(range(NC))],
                                ins=[ctx_bh[b][hl].opt()],
                                outs=[ctx_gh[b][hl].opt()])
                            yield

                # spine yields per batch: 1 + 2*(1 + 2*(16+2) + 1) = 77
                SPINE_SLOTS = 77

                qkv_gens = [qkv_work(b) for b in range(B)]
                wo_gens = [wo_work(b) for b in range(B)]

                # prologue: xc0 DMA first (critical path), then remaining
                # resident weights, then project batch 0 alone
                next(qkv_gens[0])
                load_rest_of_weights()
                f0 = Feeder(qkv_gens[0], 0, 0, 0)
                f0.gen = qkv_gens[0]
                f0.done = False
                f0.drain()

                for b in range(B):
                    feeders = []
                    if b + 1 < B:
                        feeders.append(
                            Feeder(qkv_gens[b + 1], QKV_UNITS, 2, 56))
                    if b >= 1:
                        feeders.append(
                            Feeder(wo_gens[b - 1], WO_UNITS, 30, SPINE_SLOTS - 2))
                    slot = 0
                    for _ in attn_spine(b):
                        slot += 1
                        for f in feeders:
                            f.step(slot)
                    for f in feeders:
                        f.drain()

                fN = Feeder(wo_gens[B - 1], 0, 0, 0)
                fN.gen = wo_gens[B - 1]
                fN.done = False
                fN.drain()

    nc.compile()
    return nc


def _prep_inputs(x_BLD, freqs, wqkv, wo, q_norm_w, k_norm_w):
    """Host-side sharding/layout. Returns in_maps (list of 8 dicts)."""
    x = np.asarray(x_BLD, np.float32)
    freqs = np.asarray(freqs, np.float32)
    wqkv = np.asarray(wqkv, np.float32)
    wo = np.asarray(wo, np.float32)
    qw = np.asarray(q_norm_w, np.float32)
    kw = np.asarray(k_norm_w, np.float32)

    # xQ[c] = chunk c's SBUF image: [128, a*CH + t] = x[c*CH+t, a*128+p]
    xQ = np.ascontiguousarray(
        x.reshape(T, D).astype(BF_NP)
        .reshape(B * NCPB, CH, NDT, 128).transpose(0, 3, 2, 1)
        .reshape(B * NCPB, 128, NDT * CH))
    sinT = np.ascontiguousarray(freqs[0].T)  # [D/2, L]
    cosT = np.ascontiguousarray(freqs[1].T)

    evens = 2 * np.arange(64)
    odds = evens + 1
    # ctx order after split AllGather: all even heads, then all odd heads
    woperm = np.concatenate(
        [h * HD + np.arange(HD) for h in range(0, H, 2)]
        + [h * HD + np.arange(HD) for h in range(1, H, 2)])

    in_maps = []
    for r in range(NC):
        heads = [HPC * r + hl for hl in range(HPC)]
        # q/k row order: [h0 evens, h1 evens, h0 odds, h1 odds]
        qrows = np.concatenate([h * HD + evens for h in heads]
                               + [h * HD + odds for h in heads])
        rows = np.concatenate([qrows, D + qrows, 2 * D + DQ * r + np.arange(DQ)])
        wqkvT = wqkv[rows, :].T.astype(BF_NP)          # [D, 3DQ]
        wqQ = np.ascontiguousarray(
            wqkvT.reshape(NDT, 128, 3 * DQ).transpose(1, 0, 2)
            .reshape(128, NDT * 3 * DQ))
        woT = wo[DQ * r:DQ * (r + 1), :][:, woperm].T.astype(BF_NP)
        woQ = np.ascontiguousarray(
            woT.reshape(NDT, 128, DQ).transpose(1, 0, 2)
            .reshape(128, NDT * DQ))

        tabs = {p + sfx: np.empty((HD, L), np.float32)
                for p in ("tq", "tk") for sfx in ("ce", "so", "co", "se")}
        for hl, h in enumerate(heads):
            rsl = slice(hl * 64, (hl + 1) * 64)
            cosP = cosT[h * 64:(h + 1) * 64]
            sinP = sinT[h * 64:(h + 1) * 64]
            for w, p in ((qw, "tq"), (kw, "tk")):
                w_e = w[h * HD + evens][:, None]
                w_o = w[h * HD + odds][:, None]
                tabs[p + "ce"][rsl] = w_e * cosP
                tabs[p + "so"][rsl] = w_o * sinP
                tabs[p + "co"][rsl] = w_o * cosP
                tabs[p + "se"][rsl] = w_e * sinP

        in_maps.append({
            "xQ": xQ,
            "wqQ": wqQ,
            "woQ": woQ,
            **{k: v.astype(BF_NP) for k, v in tabs.items()},
        })
    return in_maps


def _assemble(results):
    outT = np.empty((D, T), np.float32)
    for r in range(NC):
        outT[DQ * r:DQ * (r + 1)] = results[r]["outT"]
    return np.ascontiguousarray(outT.T).reshape(B, L, D)


def _install_ntff_hook():
    """The agent image's antenv lacks axon_hooks; provide the documented shim
    so run_bass_kernel_spmd(trace=True) can NTFF-profile via libaxon_pjrt."""
    try:
        import antenv.axon_hooks  # noqa: F401
        return
    except ImportError:
        pass
    import types
    hookf = None
    try:
        from trn_agent_boot.trn_boot import _ntff_profile_via_ctypes
        hookf = _ntff_profile_via_ctypes("/opt/axon/libaxon_pjrt.so")
    except Exception:
        pass
    mod = types.ModuleType("antenv.axon_hooks")
    state = {"h": hookf}
    mod.set_axon_ntff_profile_hook = lambda h: state.__setitem__("h", h)
    mod.get_axon_ntff_profile_hook = lambda: state["h"]
    sys.modules["antenv.axon_hooks"] = mod
    import antenv
    antenv.axon_hooks = mod


def kernel(x_BLD, freqs, wqkv, wo, q_norm_w, k_norm_w, _trace=False):
    from concourse.bass_utils import run_bass_kernel_spmd
    if _trace:
        _install_ntff_hook()
    if "nc" not in _CACHE:
        _CACHE["nc"] = build_nc()
    nc = _CACHE["nc"]
    in_maps = _prep_inputs(x_BLD, freqs, wqkv, wo, q_norm_w, k_norm_w)
    res = run_bass_kernel_spmd(nc, in_maps, core_ids=list(range(NC)),
                               trace=_trace)
    out = _assemble(res.results)
    if _trace:
        return out, res
    return out
